# revision 2
# baseline (speedup 1.0000x reference)
"""AffinityLoss Bass kernel v2 — fp16 pipeline for 8 TRN2 NeuronCores.

Math identical to baseline (validated in numpy, fp16 end-to-end rel err ~1.2e-3):
  loss = sum_b |S_b|^2 / (sum_b c_b^2 + 1);  S_b = sum of unit radial normals over
  selected contacts.  Selection: 10 smallest of 126 per-(kp,face) column-min
  distances, masked by dist^2 < tau^2.  Per (kp,face,iu): quadratic in k=10v
  minimized at k* = round(clamp(10*S1/A)), fv = C' - 0.2kS1 + 0.01k^2A (C' = C+1).

Key implementation points:
  - fp16 matmuls (1 cycle/row vs 4 for fp32), fp16 elementwise chain, fp32 PSUM.
  - Running min (fp16) + strict-improvement copy_predicated argmin per column.
  - A01 from sum-of-squares of edge-vector matmuls (avoids fp16-weight
    cancellation); A01B/R50B replicated 66->126 via tiny fp16 matmuls.
  - Normals scaled 8x (folded into geo weights) to keep 1/(|N|+eps) in fp16 range.
  - Engine balance: PE matmuls, Act evacs/affine decode, Pool products, DVE rest;
    input DMAs issued ahead of 2MB of constant traffic; contiguous weight layouts.
"""

import sys
import numpy as np

for _p in ("/opt/trn_rl_repo", "/root/.axon_site/_ro/trn_rl_repo"):
    if _p not in sys.path:
        sys.path.append(_p)

import concourse.bass as bass
import concourse.bacc as bacc
import concourse.mybir as mybir
import concourse.tile as tile
from concourse import bass_utils
from concourse.mybir import AluOpType as alu
from concourse.mybir import ActivationFunctionType as act

F32 = mybir.dt.float32
F16 = mybir.dt.float16
U32 = mybir.dt.uint32
AX = mybir.AxisListType

N_KP, N_C, N_F, N_IU = 21, 8, 6, 11
N_PAIR = N_KP * N_F            # 126
B_CORE = 1024
N_CORES = 8
N_TILES = B_CORE // 128
GSC = 8.0                      # geometry scale for fp16 normals
DEBUG = False

FACE = np.array([[0, 1, 2, 3], [0, 4, 2, 6], [0, 1, 4, 5],
                 [1, 3, 5, 7], [2, 3, 6, 7], [4, 5, 6, 7]])
US = np.linspace(0.0, 1.0, N_IU)

# feature rows: chunk0 (ft1) = G[kp,c] kp 0..15 (row kp*8+c)
# chunk1 (ft2): 0..39 G kp16..20 | 40..103 M[p,q] | 104..124 HH[kp] | 125 const 1


def _g_row(kp, c):
    r = kp * 8 + c
    return (0, r) if kp < 16 else (1, r - 128)


def _m_row(p, q):
    return 40 + p * 8 + q


def build_consts():
    w_s1 = np.zeros((N_IU, 2, 128, N_PAIR), np.float64)
    w_c = np.zeros((N_IU, 2, 128, N_PAIR), np.float64)
    w_aa = np.zeros((128, 66), np.float64)
    for f in range(N_F):
        F0, F1, F2, F3 = FACE[f]
        for iu in range(N_IU):
            u = US[iu]; w0 = 1.0 - u
            col66 = f * N_IU + iu
            for (p, q, s) in [(F0, F0, w0 * w0), (F0, F2, -2 * w0 * w0), (F2, F2, w0 * w0),
                              (F1, F1, u * u), (F1, F3, -2 * u * u), (F3, F3, u * u),
                              (F0, F1, 2 * u * w0), (F0, F3, -2 * u * w0),
                              (F2, F1, -2 * u * w0), (F2, F3, 2 * u * w0)]:
                w_aa[_m_row(p, q), col66] += s
            ab_terms = [(F0, F2, w0 * w0), (F0, F3, w0 * u),
                        (F2, F2, -w0 * w0), (F2, F3, -w0 * u),
                        (F1, F2, u * w0), (F1, F3, u * u),
                        (F3, F2, -u * w0), (F3, F3, -u * u)]
            bb_terms = [(F2, F2, w0 * w0), (F2, F3, 2 * w0 * u), (F3, F3, u * u)]
            for kp in range(N_KP):
                col = kp * N_F + f
                for (c, s) in [(F0, w0), (F2, -w0), (F1, u), (F3, -u)]:
                    ch, r = _g_row(kp, c)
                    w_s1[iu, ch, r, col] += s
                for (p, q, s) in ab_terms:      # S1 -= a.b
                    w_s1[iu, 1, _m_row(p, q), col] += -s
                for (c, s) in [(F2, -2 * w0), (F3, -2 * u)]:
                    ch, r = _g_row(kp, c)
                    w_c[iu, ch, r, col] += s
                for (p, q, s) in bb_terms:      # C += bb
                    w_c[iu, 1, _m_row(p, q), col] += s
                w_c[iu, 1, 104 + kp, col] += 1.0

    # packed per-iu matmul weights: [iu, chunk, 128, 0:126]=0.2*S1, [126:252]=C'
    w_s1c = np.zeros((N_IU, 2, 128, 2 * N_PAIR), np.float64)
    w_s1c[:, :, :, 0:126] = 0.2 * w_s1
    w_s1c[:, :, :, 126:252] = w_c

    # stats over ft2 rows
    w_aux = np.zeros((128, 98), np.float64)
    for i in range(4):
        for j in range(4):
            w_aux[_m_row(i, j), 66] += 1.0 / 16
            w_aux[_m_row(i + 4, j + 4), 66] += 1.0 / 16
            w_aux[_m_row(i, j + 4), 66] += -1.0 / 16
            w_aux[_m_row(i + 4, j), 66] += -1.0 / 16
    edges = [(0, 1), (1, 2), (2, 3), (3, 0), (4, 5), (5, 6), (6, 7), (7, 4)]
    for e, (i, j) in enumerate(edges):
        w_aux[_m_row(i, i), 67 + e] += 1.0
        w_aux[_m_row(j, j), 67 + e] += 1.0
        w_aux[_m_row(i, j), 67 + e] += -1.0
        w_aux[_m_row(j, i), 67 + e] += -1.0

    w_tau = np.zeros((32, 1), np.float64)
    w_tau[1:9, 0] = 0.025

    # a-edge vectors (scaled 0.1): ax[x][coord rows, (f,iu)] so A01 = sum_x ax^2
    w_ax = np.zeros((3, 128, 66), np.float64)
    for f in range(N_F):
        F0, F1, F2, F3 = FACE[f]
        for iu in range(N_IU):
            u = US[iu]; w0 = 1.0 - u
            col66 = f * N_IU + iu
            for x in range(3):
                w_ax[x, 63 + 3 * F0 + x, col66] += 0.1 * w0
                w_ax[x, 63 + 3 * F2 + x, col66] += -0.1 * w0
                w_ax[x, 63 + 3 * F1 + x, col66] += 0.1 * u
                w_ax[x, 63 + 3 * F3 + x, col66] += -0.1 * u

    # replication 66 -> 126 per iu
    w_rep = np.zeros((N_IU, 66, N_PAIR), np.float64)
    for f in range(N_F):
        for iu in range(N_IU):
            for kp in range(N_KP):
                w_rep[iu, f * N_IU + iu, kp * N_F + f] = 1.0

    # geometry basis (x, kind): kinds T1=8(c2-p1), T2=8(c3-c2), T3=8(c0-c2),
    # T4=8(c1-c3-c0+c2), DV=8(p2-p1); rows are pose-coord rows 63+3c+x of ft3
    w_geo = np.zeros((3, 5, 128, N_PAIR), np.float64)
    for f in range(N_F):
        F0, F1, F2, F3 = FACE[f]
        for x in range(3):
            row = {c: 63 + 3 * c + x for c in range(8)}
            for kp in range(N_KP):
                col = kp * N_F + f
                w_geo[x, 0, row[F2], col] += GSC
                for c in range(4):
                    w_geo[x, 0, row[c], col] += -0.25 * GSC
                w_geo[x, 1, row[F3], col] += GSC
                w_geo[x, 1, row[F2], col] -= GSC
                w_geo[x, 2, row[F0], col] += GSC
                w_geo[x, 2, row[F2], col] -= GSC
                w_geo[x, 3, row[F1], col] += GSC
                w_geo[x, 3, row[F3], col] -= GSC
                w_geo[x, 3, row[F0], col] -= GSC
                w_geo[x, 3, row[F2], col] += GSC
                for c in range(4):
                    w_geo[x, 4, row[c], col] -= 0.25 * GSC
                for c in range(4, 8):
                    w_geo[x, 4, row[c], col] += 0.25 * GSC

    f16 = np.float16
    return {
        "w_s1c": np.ascontiguousarray(
            w_s1c.astype(f16).transpose(0, 2, 1, 3)).reshape(N_IU * 128, 4 * N_PAIR),
        "w_aux": w_aux.astype(f16),
        "w_ax": np.ascontiguousarray(
            w_ax.astype(f16).transpose(1, 0, 2)).reshape(128, 3 * 66),
        "w_tau": w_tau.astype(f16),
        "w_rep": np.ascontiguousarray(
            w_rep.astype(f16).transpose(1, 0, 2)).reshape(66, N_IU * N_PAIR),
        "w_geo": np.ascontiguousarray(
            w_geo.astype(f16).reshape(15, 128, N_PAIR).transpose(1, 0, 2)).reshape(
                128, 15 * N_PAIR),
        "ident": np.eye(128, dtype=f16),
        "ones126": np.ones((126, 1), f16),
        "onesr": np.ones((1, 126), f16),
    }


def build_kernel(nc: bass.Bass):
    d = {}
    d["poses"] = nc.dram_tensor("poses", [B_CORE, 87], F32, kind="ExternalInput").ap()
    for name, shape in [("w_s1c", [N_IU * 128, 4 * N_PAIR]),
                        ("w_aux", [128, 98]), ("w_tau", [32, 1]),
                        ("w_ax", [128, 3 * 66]),
                        ("w_rep", [66, N_IU * N_PAIR]), ("w_geo", [128, 15 * N_PAIR]),
                        ("ident", [128, 128]), ("ones126", [126, 1]),
                        ("onesr", [1, 126])]:
        d[name] = nc.dram_tensor(name, shape, F16, kind="ExternalInput").ap()
    d["out"] = nc.dram_tensor("out", [2, B_CORE], F32, kind="ExternalOutput").ap()
    if DEBUG:
        for nm in ("dbg_ct", "dbg_m", "dbg_mask", "dbg_s1s", "dbg_a01b", "dbg_r50b",
                   "dbg_vc0", "dbg_wgt", "dbg_uu", "dbg_vk", "dbg_vc0pre", "dbg_inner",
                   "dbg_w", "dbg_nn"):
            d[nm] = nc.dram_tensor(nm, [126, B_CORE], F16, kind="ExternalOutput").ap()
        d["dbg_cc"] = nc.dram_tensor("dbg_cc", [126, B_CORE], F32,
                                     kind="ExternalOutput").ap()

    with tile.TileContext(nc) as tc:
        with nc.allow_low_precision(reason="fp16 pipeline validated vs reference"):
            _emit(nc, tc, d)
    return nc


def _emit(nc, tc, d):
    import contextlib
    ctx = contextlib.ExitStack()
    cpool = ctx.enter_context(tc.tile_pool(name="consts", bufs=1))
    wpool = ctx.enter_context(tc.tile_pool(name="wstream", bufs=3))
    bpool = ctx.enter_context(tc.tile_pool(name="blay", bufs=4))
    tpool = ctx.enter_context(tc.tile_pool(name="tlay", bufs=1))
    colpool = ctx.enter_context(tc.tile_pool(name="col", bufs=4))

    # ------------- input first: poses DMAs ahead of all const traffic ----------
    pbpool = ctx.enter_context(tc.tile_pool(name="pbl", bufs=1))
    pbs = []
    for t in range(N_TILES):
        pbt = pbpool.tile([128, 87], F32, tag=f"pb{t}")
        nc.sync.dma_start(out=pbt[:, :], in_=d["poses"][t * 128:(t + 1) * 128, :])
        pbs.append(pbt)

    # ------------- resident consts (emitted after input loads) -------------
    ident = cpool.tile([128, 128], F16, tag="ident")
    nc.sync.dma_start(out=ident[:, :], in_=d["ident"])
    w_aux = cpool.tile([128, 98], F16, tag="w_aux")
    nc.sync.dma_start(out=w_aux[:, :], in_=d["w_aux"])
    w_ax = cpool.tile([128, 3 * 66], F16, tag="w_ax")
    nc.sync.dma_start(out=w_ax[:, :], in_=d["w_ax"])

    w_tau = cpool.tile([32, 1], F16, tag="w_tau")
    nc.sync.dma_start(out=w_tau[:, :], in_=d["w_tau"])
    ones126 = cpool.tile([126, 1], F16, tag="ones126")
    nc.sync.dma_start(out=ones126[:, :], in_=d["ones126"])
    onesr = cpool.tile([1, 126], F16, tag="onesr")
    nc.sync.dma_start(out=onesr[:, :], in_=d["onesr"])

    w_s1cR = cpool.tile([128, N_IU * 4 * N_PAIR], F16, tag="w_s1cR")
    for iu_ in range(N_IU):
        nc.sync.dma_start(
            out=w_s1cR[:, iu_ * 4 * N_PAIR:(iu_ + 1) * 4 * N_PAIR],
            in_=d["w_s1c"].rearrange("(i k) m -> i k m", i=N_IU)[iu_])
    w_geo = cpool.tile([128, 15 * N_PAIR], F16, tag="w_geo")
    for ch in range(5):
        nc.sync.dma_start(out=w_geo[:, ch * 3 * N_PAIR:(ch + 1) * 3 * N_PAIR],
                          in_=d["w_geo"][:, ch * 3 * N_PAIR:(ch + 1) * 3 * N_PAIR])
    ft1 = tpool.tile([128, B_CORE], F16, tag="ft1")
    ft2 = tpool.tile([128, B_CORE], F16, tag="ft2")
    ft3 = tpool.tile([128, B_CORE], F16, tag="ft3")

    # ------------- B-stage: features + transpose (fp16) -------------
    with tc.tile_pool(name="psA", bufs=4, space="PSUM") as psA:
        for t in range(N_TILES):
            cs = slice(t * 128, (t + 1) * 128)
            pb = pbs[t]
            pb16 = bpool.tile([128, 128], F16, tag="poseb16")
            nc.vector.memset(pb16[:, 87:128], 0.0)
            nc.vector.tensor_copy(pb16[:, 0:87], pb[:, :])
            h_ap = pb16[:, 0:63].rearrange("p (k x) -> p k x", x=3)
            o_ap = pb16[:, 63:87].rearrange("p (c x) -> p c x", x=3)
            fb = bpool.tile([128, 256], F16, tag="featb")
            nc.vector.memset(fb[:, 253:256], 0.0)
            sc1 = bpool.tile([128, 504], F16, tag="sc1")
            nc.vector.tensor_tensor(sc1[:, :].rearrange("p (k c x) -> p k c x", c=8, x=3),
                                    h_ap.unsqueeze(2).to_broadcast([128, 21, 8, 3]),
                                    o_ap.unsqueeze(1).to_broadcast([128, 21, 8, 3]),
                                    op=alu.mult)
            nc.vector.tensor_reduce(fb[:, 0:168].rearrange("p (k c) -> p k c", c=8),
                                    sc1[:, :].rearrange("p (k c x) -> p k c x", c=8, x=3),
                                    axis=AX.X, op=alu.add)
            sc2 = bpool.tile([128, 192], F16, tag="sc2")
            nc.gpsimd.tensor_tensor(sc2[:, :].rearrange("p (a b x) -> p a b x", b=8, x=3),
                                    o_ap.unsqueeze(2).to_broadcast([128, 8, 8, 3]),
                                    o_ap.unsqueeze(1).to_broadcast([128, 8, 8, 3]),
                                    op=alu.mult)
            nc.vector.tensor_reduce(fb[:, 168:232].rearrange("p (a b) -> p a b", b=8),
                                    sc2[:, :].rearrange("p (a b x) -> p a b x", b=8, x=3),
                                    axis=AX.X, op=alu.add)
            sc3 = bpool.tile([128, 63], F16, tag="sc3")
            nc.gpsimd.tensor_tensor(sc3[:, :].rearrange("p (k x) -> p k x", x=3),
                                    h_ap, h_ap, op=alu.mult)
            nc.vector.tensor_reduce(fb[:, 232:253].rearrange("p k -> p k"),
                                    sc3[:, :].rearrange("p (k x) -> p k x", x=3),
                                    axis=AX.X, op=alu.add)
            for (src, dst) in ((fb[:, 0:128], ft1), (fb[:, 128:256], ft2),
                               (pb16[:, :], ft3)):
                pt = psA.tile([128, 128], F16, tag="tpose")
                nc.tensor.transpose(out=pt[:, :], in_=src, identity=ident[:, :])
                nc.scalar.activation(dst[:, cs], pt[:, :], act.Copy)

    # ------------- P-stage: A01, R50, stats, tau2, rdvn2p -------------
    a01_16 = tpool.tile([66, B_CORE], F16, tag="a01_16")
    a01_32 = tpool.tile([66, B_CORE], F32, tag="a01_32")
    r50_16 = tpool.tile([66, B_CORE], F16, tag="r50_16")
    stats16 = tpool.tile([32, B_CORE], F16, tag="stats16")
    dvn32 = tpool.tile([1, B_CORE], F32, tag="dvn32")
    tau2t = tpool.tile([1, B_CORE], F16, tag="tau2t")
    rdb = tpool.tile([126, B_CORE], F16, tag="rdb")
    with tc.tile_pool(name="psP", bufs=1, space="PSUM") as psP:
        for h in range(2):
            bs = slice(h * 512, (h + 1) * 512)
            axs = []
            for x in range(3):
                ps = psP.tile([66, 512], F32, tag=f"ps_ax{x}")
                nc.tensor.matmul(ps[:, :], lhsT=w_ax[:, x * 66:(x + 1) * 66],
                                 rhs=ft3[:, bs], start=True, stop=True)
                axt = tpool.tile([66, 512], F16, tag=f"axt{x}_{h}")
                nc.scalar.activation(axt[:, :], ps[:, :], act.Copy)
                axs.append(axt)
            sq0 = tpool.tile([66, 512], F16, tag=f"sq0_{h}")
            nc.vector.tensor_tensor(sq0[:, :], axs[0][:, :], axs[0][:, :], op=alu.mult)
            sq1 = tpool.tile([66, 512], F16, tag=f"sq1_{h}")
            nc.vector.tensor_tensor(sq1[:, :], axs[1][:, :], axs[1][:, :], op=alu.mult)
            nc.vector.tensor_tensor(sq0[:, :], sq0[:, :], sq1[:, :], op=alu.add)
            nc.vector.tensor_tensor(sq1[:, :], axs[2][:, :], axs[2][:, :], op=alu.mult)
            nc.vector.tensor_tensor(a01_16[:, bs], sq0[:, :], sq1[:, :], op=alu.add)
            nc.vector.tensor_copy(a01_32[:, bs], a01_16[:, bs])
            ps2 = psP.tile([32, 512], F32, tag="ps_stat")
            nc.tensor.matmul(ps2[:, :], lhsT=w_aux[:, 66:98], rhs=ft2[:, bs],
                             start=True, stop=True)
            nc.scalar.activation(stats16[:, bs], ps2[:, :], act.Sqrt)
            nc.scalar.activation(dvn32[:, bs], ps2[0:1, :], act.Sqrt)
        # R50 = min(0.5/(A01+1e-8), 6e4)
        rc = tpool.tile([66, B_CORE], F32, tag="rc")
        nc.vector.tensor_scalar(rc[:, :], a01_32[:, :], 1e-8, None, op0=alu.add)
        nc.vector.reciprocal_approx_fast(out=rc[:, :], in_=rc[:, :])
        nc.vector.tensor_scalar(r50_16[:, :], rc[:, :], 0.5, 60000.0,
                                op0=alu.mult, op1=alu.min)
        # tau2+1-1e-6
        for h in range(2):
            bs = slice(h * 512, (h + 1) * 512)
            ps3 = psP.tile([1, 512], F32, tag="ps_tau")
            nc.tensor.matmul(ps3[:, :], lhsT=w_tau[:, :], rhs=stats16[:, bs],
                             start=True, stop=True)
            tau32 = tpool.tile([1, 512], F32, tag=f"tau32_{h}")
            nc.scalar.activation(tau32[:, :], ps3[:, :], act.Square)
            nc.vector.tensor_scalar(tau2t[:, bs], tau32[:, :], -1e-6, None,
                                    op0=alu.add)
        # rdvn2p = 1/(64*(dvn+1e-5)^2), replicated to 126 rows (fp16)
        rv = tpool.tile([1, B_CORE], F32, tag="rv")
        nc.vector.tensor_scalar(rv[:, :], dvn32[:, :], 1e-5, None, op0=alu.add)
        nc.vector.reciprocal_approx_fast(out=rv[:, :], in_=rv[:, :])
        nc.vector.tensor_tensor(rv[:, :], rv[:, :], rv[:, :], op=alu.mult)
        rv16 = tpool.tile([1, B_CORE], F16, tag="rv16")
        nc.vector.tensor_scalar(rv16[:, :], rv[:, :], 1.0 / 64.0, None, op0=alu.mult)
        for h in range(2):
            bs = slice(h * 512, (h + 1) * 512)
            ps4 = psP.tile([126, 512], F32, tag="ps_rd")
            nc.tensor.matmul(ps4[:, :], lhsT=onesr[:, :], rhs=rv16[:, bs],
                             start=True, stop=True)
            nc.scalar.activation(rdb[:, bs], ps4[:, :], act.Copy)

    # ------------- column loop: fp16 running min + tagged argmin -------------
    m16 = tpool.tile([126, B_CORE], F16, tag="m16")
    nc.vector.memset(m16[:, :], 60000.0)
    ct16 = tpool.tile([126, B_CORE], F16, tag="ct16")
    nc.vector.memset(ct16[:, :], 60000.0)

    with tc.tile_pool(name="psC", bufs=2, space="PSUM") as psC:
        for iu in range(N_IU):
            wt = w_s1cR[:, iu * 4 * N_PAIR:(iu + 1) * 4 * N_PAIR]
            rep_t = wpool.tile([66, N_PAIR], F16, tag="rep")
            nc.sync.dma_start(out=rep_t[:, :],
                              in_=d["w_rep"][:, iu * N_PAIR:(iu + 1) * N_PAIR])
            rep = rep_t[:, :]
            for h in range(2):
                bs = slice(h * 512, (h + 1) * 512)
                psS1 = psC.tile([126, 512], F32, tag="psS1")
                nc.tensor.matmul(psS1[:, :], lhsT=wt[:, 0:126], rhs=ft1[:, bs],
                                 start=True, stop=False)
                nc.tensor.matmul(psS1[:, :], lhsT=wt[:, 252:378], rhs=ft2[:, bs],
                                 start=False, stop=True)
                psCc = psC.tile([126, 512], F32, tag="psCc")
                nc.tensor.matmul(psCc[:, :], lhsT=wt[:, 126:252], rhs=ft1[:, bs],
                                 start=True, stop=False)
                nc.tensor.matmul(psCc[:, :], lhsT=wt[:, 378:504], rhs=ft2[:, bs],
                                 start=False, stop=True)
                psA01 = psC.tile([126, 512], F32, tag="psA01")
                nc.tensor.matmul(psA01[:, :], lhsT=rep, rhs=a01_16[:, bs],
                                 start=True, stop=True)
                psR50 = psC.tile([126, 512], F32, tag="psR50")
                nc.tensor.matmul(psR50[:, :], lhsT=rep, rhs=r50_16[:, bs],
                                 start=True, stop=True)

                s1s = colpool.tile([126, 512], F16, tag="s1s")
                nc.scalar.activation(s1s[:, :], psS1[:, :], act.Copy)

                if DEBUG and iu == 0:
                    dcc = colpool.tile([126, 512], F32, tag="dcc")
                    nc.scalar.activation(dcc[:, :], psCc[:, :], act.Copy)
                    nc.sync.dma_start(out=d["dbg_cc"][:, h * 512:(h + 1) * 512],
                                      in_=dcc[:, :])
                    da = colpool.tile([126, 512], F16, tag="da")
                    nc.scalar.activation(da[:, :], psA01[:, :], act.Copy)
                    dr = colpool.tile([126, 512], F16, tag="dr")
                    nc.scalar.activation(dr[:, :], psR50[:, :], act.Copy)
                    for nm, t_ in (("dbg_s1s", s1s), ("dbg_a01b", da), ("dbg_r50b", dr)):
                        nc.sync.dma_start(out=d[nm][:, h * 512:(h + 1) * 512], in_=t_[:, :])
                kc = colpool.tile([126, 512], F16, tag="kc")
                nc.vector.tensor_tensor(kc[:, :], s1s[:, :], psR50[:, :], op=alu.mult)
                nc.vector.tensor_scalar(kc[:, :], kc[:, :], 0.0, None, op0=alu.max)
                nc.vector.tensor_scalar(kc[:, :], kc[:, :], 10.0, None, op0=alu.min)
                nc.vector.tensor_scalar(kc[:, :], kc[:, :], 1024.0, None, op0=alu.add)
                kk = colpool.tile([126, 512], F16, tag="kk")
                nc.scalar.activation(kk[:, :], kc[:, :], act.Copy, bias=-1024.0)
                kA = colpool.tile([126, 512], F16, tag="kA")
                nc.vector.tensor_tensor(kA[:, :], kk[:, :], psA01[:, :], op=alu.mult)
                nc.gpsimd.tensor_tensor(kA[:, :], kA[:, :], s1s[:, :], op=alu.subtract)
                r = colpool.tile([126, 512], F16, tag="r")
                nc.gpsimd.tensor_tensor(r[:, :], kk[:, :], kA[:, :], op=alu.mult)
                cc16 = colpool.tile([126, 512], F16, tag="cc16")
                nc.scalar.activation(cc16[:, :], psCc[:, :], act.Copy)
                nc.gpsimd.tensor_tensor(r[:, :], r[:, :], cc16[:, :], op=alu.add)
                ki = colpool.tile([126, 512], F16, tag="ki")
                nc.scalar.activation(ki[:, :], kc[:, :], act.Copy,
                                     bias=float(16 * iu - 1024))
                # strict-improvement replace (first-occurrence tie semantics)
                cond = colpool.tile([126, 512], mybir.dt.uint16, tag="cond")
                nc.vector.tensor_tensor(cond[:, :], r[:, :], m16[:, bs], op=alu.is_lt)
                nc.vector.tensor_tensor(m16[:, bs], m16[:, bs], r[:, :], op=alu.min)
                nc.vector.copy_predicated(out=ct16[:, bs], mask=cond[:, :],
                                          data=ki[:, :])

    # ------------- decode -------------
    iuf = tpool.tile([126, B_CORE], F16, tag="iuf")
    nc.scalar.activation(iuf[:, :], ct16[:, :], act.Copy, bias=1024.66875, scale=0.0625)
    nc.scalar.activation(iuf[:, :], iuf[:, :], act.Copy, bias=-1025.0)
    z16 = tpool.tile([126, B_CORE], F16, tag="z16")
    nc.scalar.activation(z16[:, :], iuf[:, :], act.Copy, scale=-16.0)
    kst = tpool.tile([126, B_CORE], F16, tag="kst")
    nc.vector.tensor_tensor(kst[:, :], ct16[:, :], z16[:, :], op=alu.add)
    uu = tpool.tile([126, B_CORE], F16, tag="uu")
    nc.scalar.activation(uu[:, :], iuf[:, :], act.Copy, scale=0.1)
    vk = tpool.tile([126, B_CORE], F16, tag="vk")
    nc.scalar.activation(vk[:, :], kst[:, :], act.Copy, scale=0.1)
    uv = tpool.tile([126, B_CORE], F16, tag="uv")
    nc.vector.tensor_tensor(uv[:, :], uu[:, :], vk[:, :], op=alu.mult)
    if DEBUG:
        nc.sync.dma_start(out=d["dbg_ct"], in_=ct16[:, :])
        nc.sync.dma_start(out=d["dbg_m"], in_=m16[:, :])

    # ------------- selection (B-layout) interleaved with geometry -------------
    mask_t = tpool.tile([126, B_CORE], F16, tag="mask_t")
    dv8 = []
    vc8 = []
    for x in range(3):
        dv8x = tpool.tile([126, B_CORE], F16, tag=f"dv8_{x}")
        dv8.append(dv8x)
        vc8x = tpool.tile([126, B_CORE], F16, tag=f"vc8_{x}")
        vc8.append(vc8x)

    def emit_sel_tile(psX, t):
        cs = slice(t * 128, (t + 1) * 128)
        pt = psX.tile([128, 126], F16, tag="tp")
        nc.tensor.transpose(out=pt[:, 0:126], in_=m16[:, cs],
                            identity=ident[0:126, 0:126])
        mb = bpool.tile([128, 126], F32, tag="mb")
        nc.scalar.activation(mb[:, :], pt[:, 0:126], act.Copy)
        ptt = psX.tile([128, 8], F16, tag="tp2")
        nc.tensor.transpose(out=ptt[:, 0:1], in_=tau2t[:, cs],
                            identity=ident[0:1, 0:1])
        tb = bpool.tile([128, 1], F32, tag="tb")
        nc.scalar.activation(tb[:, :], ptt[:, 0:1], act.Copy)

        neg = bpool.tile([128, 126], F32, tag="neg")
        nc.vector.tensor_scalar(neg[:, :], mb[:, :], -1.0, None, op0=alu.mult)
        v8a = bpool.tile([128, 8], F32, tag="v8a")
        nc.vector.max(out=v8a[:, :], in_=neg[:, :])
        negr = bpool.tile([128, 126], F32, tag="negr")
        nc.vector.match_replace(out=negr[:, :], in_to_replace=v8a[:, :],
                                in_values=neg[:, :], imm_value=-3.0e38)
        v8b = bpool.tile([128, 8], F32, tag="v8b")
        nc.vector.max(out=v8b[:, :], in_=negr[:, :])
        nc.vector.memset(v8b[:, 2:8], -2.9e38)
        m1 = bpool.tile([128, 126], F32, tag="m1")
        nc.vector.match_replace(out=m1[:, :], in_to_replace=v8a[:, :],
                                in_values=neg[:, :], imm_value=1.0e38)
        m2 = bpool.tile([128, 126], F32, tag="m2")
        nc.vector.match_replace(out=m2[:, :], in_to_replace=v8b[:, :],
                                in_values=m1[:, :], imm_value=1.0e38)
        sel = bpool.tile([128, 126], F32, tag="sel")
        nc.vector.tensor_scalar(sel[:, :], m2[:, :], 9.0e37, None, op0=alu.is_ge)
        tcmp = bpool.tile([128, 126], F32, tag="tcmp")
        nc.vector.tensor_scalar(tcmp[:, :], mb[:, :], tb[:, 0:1], None, op0=alu.is_lt)
        mask = bpool.tile([128, 126], F16, tag="mask")
        nc.vector.tensor_tensor(mask[:, :], sel[:, :], tcmp[:, :], op=alu.mult)
        ptm = psX.tile([126, 128], F16, tag="tp")
        nc.tensor.transpose(out=ptm[:, :], in_=mask[:, :], identity=ident[:, :])
        nc.scalar.activation(mask_t[:, cs], ptm[:, :], act.Copy)

    def emit_geo(psX, x, h):
        bs = slice(h * 512, (h + 1) * 512)
        if x == 0 and h == 0:
            for xx in range(3):
                for hh in range(2):
                    bss = slice(hh * 512, (hh + 1) * 512)
                    ps = psX.tile([126, 512], F32, tag="psg3")
                    nc.tensor.matmul(
                        ps[:, :],
                        lhsT=w_geo[:, (xx * 5 + 4) * N_PAIR:(xx * 5 + 5) * N_PAIR],
                        rhs=ft3[:, bss], start=True, stop=True)
                    nc.scalar.activation(dv8[xx][:, bss], ps[:, :], act.Copy)
        pst = []
        for g in range(4):
            ps = psX.tile([126, 512], F32, tag=f"psg{g}")
            nc.tensor.matmul(ps[:, :],
                             lhsT=w_geo[:, (x * 5 + g) * N_PAIR:(x * 5 + g + 1) * N_PAIR],
                             rhs=ft3[:, bs], start=True, stop=True)
            pst.append(ps)
        t1x, t2x, t3x, t4x = pst
        q1 = colpool.tile([126, 512], F16, tag="gq1")
        nc.vector.tensor_tensor(q1[:, :], uu[:, bs], t2x[:, :], op=alu.mult)
        q2 = colpool.tile([126, 512], F16, tag="gq2")
        nc.vector.tensor_tensor(q2[:, :], vk[:, bs], t3x[:, :], op=alu.mult)
        q3 = colpool.tile([126, 512], F16, tag="gq3")
        nc.vector.tensor_tensor(q3[:, :], uv[:, bs], t4x[:, :], op=alu.mult)
        nc.gpsimd.tensor_tensor(q1[:, :], q1[:, :], q2[:, :], op=alu.add)
        nc.gpsimd.tensor_tensor(q1[:, :], q1[:, :], q3[:, :], op=alu.add)
        nc.vector.tensor_tensor(vc8[x][:, bs], q1[:, :], t1x[:, :], op=alu.add)

    with tc.tile_pool(name="psX", bufs=1, space="PSUM") as psX:
        geo_iters = [(x, h) for x in range(3) for h in range(2)]
        for i in range(8):
            emit_sel_tile(psX, i)
            if i < 6:
                emit_geo(psX, *geo_iters[i])

    if DEBUG:
        nc.sync.dma_start(out=d["dbg_mask"], in_=mask_t[:, :])
        nc.sync.dma_start(out=d["dbg_uu"], in_=uu[:, :])
        nc.sync.dma_start(out=d["dbg_vk"], in_=vk[:, :])
        nc.sync.dma_start(out=d["dbg_vc0pre"], in_=vc8[0][:, :])

    if DEBUG:
        nc.sync.dma_start(out=d["dbg_uu"], in_=uu[:, :])
        nc.sync.dma_start(out=d["dbg_vk"], in_=vk[:, :])
        nc.sync.dma_start(out=d["dbg_vc0pre"], in_=vc8[0][:, :])

    # ------------- normals + final contraction -------------
    p0 = tpool.tile([126, B_CORE], F16, tag="axt0_0")
    nc.vector.tensor_tensor(p0[:, :], vc8[0][:, :], dv8[0][:, :], op=alu.mult)
    p1 = tpool.tile([126, B_CORE], F16, tag="axt1_0")
    nc.gpsimd.tensor_tensor(p1[:, :], vc8[1][:, :], dv8[1][:, :], op=alu.mult)
    i01 = tpool.tile([126, B_CORE], F16, tag="axt2_0")
    nc.vector.tensor_tensor(i01[:, :], p0[:, :], p1[:, :], op=alu.add)
    p2 = tpool.tile([126, B_CORE], F16, tag="axt0_1")
    nc.gpsimd.tensor_tensor(p2[:, :], vc8[2][:, :], dv8[2][:, :], op=alu.mult)
    inner8 = tpool.tile([126, B_CORE], F16, tag="sq0_0")
    nc.vector.tensor_tensor(inner8[:, :], i01[:, :], p2[:, :], op=alu.add)
    wq = tpool.tile([126, B_CORE], F16, tag="sq1_0")
    nc.vector.tensor_tensor(wq[:, :], inner8[:, :], rdb[:, :], op=alu.mult)
    if DEBUG:
        nc.sync.dma_start(out=d["dbg_inner"], in_=inner8[:, :])
        nc.sync.dma_start(out=d["dbg_w"], in_=wq[:, :])
    nn8 = tpool.tile([126, B_CORE], F16, tag="sq0_1")
    for x in range(3):
        wd = tpool.tile([126, B_CORE], F16, tag="axt1_1")
        eng = nc.vector if x == 0 else nc.gpsimd
        eng.tensor_tensor(wd[:, :], wq[:, :], dv8[x][:, :], op=alu.mult)
        nc.vector.tensor_tensor(vc8[x][:, :], vc8[x][:, :], wd[:, :], op=alu.subtract)
        sq = tpool.tile([126, B_CORE], F16, tag="axt2_1")
        (nc.gpsimd if x == 1 else nc.vector).tensor_tensor(
            sq[:, :], vc8[x][:, :], vc8[x][:, :], op=alu.mult)
        if x == 0:
            nc.vector.tensor_copy(nn8[:, :], sq[:, :])
        else:
            nc.vector.tensor_tensor(nn8[:, :], nn8[:, :], sq[:, :], op=alu.add)
    if DEBUG:
        nc.sync.dma_start(out=d["dbg_nn"], in_=nn8[:, :])
    sq32 = tpool.tile([126, B_CORE], F32, tag="rc")
    nc.scalar.activation(sq32[:, :], nn8[:, :], act.Sqrt)
    nc.vector.tensor_scalar(sq32[:, :], sq32[:, :], 8e-5, None, op0=alu.add)
    rn32 = tpool.tile([126, B_CORE], F32, tag="a01_32")
    nc.vector.reciprocal_approx_fast(out=rn32[:, :], in_=sq32[:, :])
    rn16 = tpool.tile([126, B_CORE], F16, tag="rn16")
    nc.scalar.activation(rn16[:, :], rn32[:, :], act.Copy)
    wgt = tpool.tile([126, B_CORE], F16, tag="wgt")
    nc.vector.tensor_tensor(wgt[:, :], mask_t[:, :], rn16[:, :], op=alu.mult)

    if DEBUG:
        nc.sync.dma_start(out=d["dbg_vc0"], in_=vc8[0][:, :])
        nc.sync.dma_start(out=d["dbg_wgt"], in_=wgt[:, :])
    num_t = tpool.tile([1, B_CORE], F32, tag="num_t")
    den_t = tpool.tile([1, B_CORE], F32, tag="den_t")
    sx = []
    for x in range(3):
        sxx = tpool.tile([1, B_CORE], F32, tag=f"sx{x}")
        sx.append(sxx)
    contrib = tpool.tile([126, B_CORE], F16, tag="sq1_1")
    with tc.tile_pool(name="psF", bufs=2, space="PSUM") as psF:
        for x in range(3):
            nc.vector.tensor_tensor(contrib[:, :], vc8[x][:, :], wgt[:, :], op=alu.mult)
            for h in range(2):
                bs = slice(h * 512, (h + 1) * 512)
                ps = psF.tile([1, 512], F32, tag="psx")
                nc.tensor.matmul(ps[:, :], lhsT=ones126[:, :], rhs=contrib[:, bs],
                                 start=True, stop=True)
                nc.scalar.activation(sx[x][:, bs], ps[:, :], act.Copy)
        for h in range(2):
            bs = slice(h * 512, (h + 1) * 512)
            ps = psF.tile([1, 512], F32, tag="psc")
            nc.tensor.matmul(ps[:, :], lhsT=ones126[:, :], rhs=mask_t[:, bs],
                             start=True, stop=True)
            nc.scalar.activation(den_t[:, bs], ps[:, :], act.Square)
    nc.vector.tensor_tensor(num_t[:, :], sx[0][:, :], sx[0][:, :], op=alu.mult)
    for x in (1, 2):
        nc.vector.tensor_tensor(sx[x][:, :], sx[x][:, :], sx[x][:, :], op=alu.mult)
        nc.vector.tensor_tensor(num_t[:, :], num_t[:, :], sx[x][:, :], op=alu.add)
    nc.sync.dma_start(out=d["out"][0:1, :], in_=num_t[:, :])
    nc.sync.dma_start(out=d["out"][1:2, :], in_=den_t[:, :])
    ctx.close()


# ------------------------------------------------------------------ host side

_CACHE = {}


def _get_compiled():
    if "nc" not in _CACHE:
        nc = bacc.Bacc("TRN2", target_bir_lowering=False, debug=False,
                       enable_asserts=False, num_devices=N_CORES)
        build_kernel(nc)
        nc.compile()
        _CACHE["nc"] = nc
    return _CACHE["nc"]


def kernel(poses: np.ndarray) -> np.ndarray:
    poses = np.asarray(poses, dtype=np.float32)
    bs = poses.shape[0]
    assert bs == B_CORE * N_CORES, f"expected {B_CORE * N_CORES}, got {bs}"
    consts = build_consts()
    nc = _get_compiled()
    in_maps = []
    for c in range(N_CORES):
        m = {"poses": poses[c * B_CORE:(c + 1) * B_CORE].reshape(B_CORE, 87).copy()}
        m.update(consts)
        in_maps.append(m)
    res = bass_utils.run_bass_kernel_spmd(nc, in_maps, core_ids=list(range(N_CORES)))
    num = 0.0
    den = 0.0
    for c in range(N_CORES):
        o = res.results[c]["out"]
        num += o[0, :].sum(dtype=np.float64)
        den += o[1, :].sum(dtype=np.float64)
    return np.float32(num / (den + 1.0))
